# revision 40
# baseline (speedup 1.0000x reference)
"""AttentionBlock (InstanceNorm + single-head self-attention over 64x64 pixels
+ residual) on 8 Trainium2 NeuronCores.

Sharding: core = (batch b = core//2, query-half h = core%2). Each core gets the
full 512x4096 plane of its batch (columns rolled so its 2048 query pixels are
columns 0..2047), computes norm + K/V for all 4096 pixels and Q for its 2048,
runs softmax(Q^T K / sqrt(C)) V and the output projection for its half, and
returns a [512, 2048] shard. No collectives.

Numerics / structure:
- x ships once as fp8e4m3 in DoubleRow pair layout (plus a bf16 copy of the
  query half for the residual). bp2 = wp@bv + bp is folded into x on the host:
  stats shift by bp2 but the normalized tensor is invariant, and the residual
  needs x + bp2 anyway.
- InstanceNorm is folded into the projections: q/v = (W*rstd) @ x + (b + W@nmb)
  with nmb = -mu*rstd. The fp8 weights are prescaled by 8 on the host so the
  device-side rstd scaling lands in fp8's normal range; the 8x factors cancel
  exactly via powers of two (ones=8.0 in the Z matmul, exp scale /64, wpcv
  drain scale 0.125).
- K needs NO bias: adding a j-independent vector to every key shifts each
  query's logit row by a constant, which softmax cancels exactly.
- q/k/v projections are fp8 DoubleRow matmuls (256-deep contraction per
  instruction). QK^T and exp()V likewise. PSUM drains are 1024-wide (2-bank
  PSUM tiles), spread across ACT/DVE/GpSimd.
- Softmax: logitsT[j,i] pair tiles get a single exp() per jt-pair; the
  denominator accumulates on DVE and GpSimd in parallel, partition-reduced by
  one ones(=8) matmul; 1/Z is fused into the U drain (o = U * rzb).
- Epilogue: xr' = xr + wpcv on ACT as the residual lands; y = psP + xr' is a
  single DVE add per tile.
"""

import numpy as np
import ml_dtypes

import concourse.bass as bass
import concourse.mybir as mybir
import concourse.tile as tile
from concourse import bacc
from concourse import bass_utils

C = 512          # channels
HW = 4096        # pixels per plane (64*64)
NQ = 2048        # query pixels per core
B = 4            # batch
N_CORES = 8
CT = C // 128    # channel tiles (4)
JT = HW // 128   # key tiles on partitions (32)
JP = JT // 2     # key tile pairs for DoubleRow (16)
IB = NQ // 512   # query i-blocks of 512 (4)
EPS = 1e-5
WS = 8.0         # host-side fp8 weight prescale (power of two, cancels)
SCALE8 = 1.0 / (WS * WS * np.sqrt(np.float32(C)))
EXP_OFF = -5.0   # exp offset; cancels in U/Z, keeps fp8 exp in range

F32 = mybir.dt.float32
BF16 = mybir.dt.bfloat16
FP8 = mybir.dt.float8e4
AF = mybir.ActivationFunctionType
DR = mybir.MatmulPerfMode.DoubleRow


def build_nc():
    nc = bacc.Bacc("TRN2", target_bir_lowering=False, debug=False,
                   num_devices=N_CORES)
    # x8[p, g, j, n] = fp8(x[g*256 + j*128 + p, n] + bp2)
    x8 = nc.dram_tensor("x8", [128, 2, 2, HW], FP8, kind="ExternalInput").ap()
    # residual (query half only): xr[p, ct, i] = bf16(x[ct*128+p, i] + bp2)
    xr = nc.dram_tensor("xr", [128, CT, NQ], BF16, kind="ExternalInput").ap()
    # pair-layout weights: w_p[p, g, j, o] = 8*w[o, g*256 + j*128 + p]
    wqp = nc.dram_tensor("wqp", [128, 2, 2, C], BF16, kind="ExternalInput").ap()
    wkp = nc.dram_tensor("wkp", [128, 2, 2, C], BF16, kind="ExternalInput").ap()
    wvp = nc.dram_tensor("wvp", [128, 2, 2, C], BF16, kind="ExternalInput").ap()
    # bq8[p, ct] = 8*bq[ct*128 + p]
    bq8 = nc.dram_tensor("bq8", [128, CT], F32, kind="ExternalInput").ap()
    out = nc.dram_tensor("out", [C, NQ], F32, kind="ExternalOutput").ap()

    with tile.TileContext(nc) as tc:
        build_graph(tc, x8, xr, wqp, wkp, wvp, bq8, out)
    nc.compile()
    return nc


def build_graph(tc, x8, xr, wqp, wkp, wvp, bq8, out):
    nc = tc.nc
    with (
        tc.tile_pool(name="const", bufs=1) as const,
        tc.tile_pool(name="qk", bufs=1) as qkp,
        tc.tile_pool(name="vt", bufs=1) as vtp,
        tc.tile_pool(name="bc", bufs=1) as bcp,
    ):
        # ---- input DMAs on the SP + GpSimd queues (ACT stays free) ----
        x8_sb = const.tile([128, 2, 2, HW], FP8, tag="x8", name="x8_sb")
        # per-ct slices; ct = 2*g + j
        def x8s(ct):
            return x8_sb[:, ct // 2, ct % 2, :]
        # ct0 with a small first chunk (stats start sooner), ct2/ct3 whole
        for lo, hi in ((0, 1024), (1024, 2048), (2048, 4096)):
            nc.sync.dma_start(out=x8s(0)[:, lo:hi],
                              in_=x8[:, 0, 0, lo:hi])
        for h in range(2):
            nc.gpsimd.dma_start(out=x8s(1)[:, h * 2048:(h + 1) * 2048],
                                in_=x8[:, 0, 1, h * 2048:(h + 1) * 2048])
        nc.sync.dma_start(out=x8s(2), in_=x8[:, 1, 0, :])
        nc.gpsimd.dma_start(out=x8s(3), in_=x8[:, 1, 1, :])

        w_p = {}
        for wname, wap, q in (("wk", wkp, nc.sync), ("wq", wqp, nc.gpsimd),
                              ("wv", wvp, nc.sync)):
            t = const.tile([128, 2, 2, C], BF16, tag=wname, name=wname)
            q.dma_start(out=t, in_=wap)
            w_p[wname] = t
        bq8_sb = const.tile([128, CT], F32, tag="bq8", name="bq8_sb")
        nc.gpsimd.dma_start(out=bq8_sb, in_=bq8)

        FP16 = mybir.dt.float16
        ones8_sb = const.tile([128, 128], FP16, tag="ones8", name="ones8")
        nc.vector.memset(ones8_sb, WS)
        eps_sb = const.tile([128, 1], F32, tag="eps", name="eps")
        nc.vector.memset(eps_sb, EPS)
        expoff_sb = const.tile([128, 1], F32, tag="expoff", name="expoff")
        nc.vector.memset(expoff_sb, EXP_OFF)

        # persistent activations (fp8 DoubleRow pair layouts)
        q_sb = [qkp.tile([128, 2, NQ], FP8, tag=f"q{g}", name=f"q{g}")
                for g in range(2)]
        k_sb = [qkp.tile([128, 2, HW], FP8, tag=f"k{g}", name=f"k{g}")
                for g in range(2)]
        vT_sb = [vtp.tile([128, 2, C], FP8, tag=f"vT{jtp}", name=f"vT{jtp}")
                 for jtp in range(JP)]

        # scaled fp8 weights ws8[wname][g] = w_p * rstd (per input channel)
        ws8 = {wn: [bcp.tile([128, 2, C], FP8, tag=f"{wn}8{g}",
                             name=f"{wn}8{g}") for g in range(2)]
               for wn in ("wq", "wk", "wv")}
        negmu8 = [bcp.tile([128, 2, 1], FP8, tag=f"nmu{g}", name=f"negmu8{g}")
                  for g in range(2)]
        qbias_sb = [bcp.tile([128, 1], F32, tag=f"qb{mt}", name=f"qb{mt}")
                    for mt in range(CT)]
        wpcv_sb = [bcp.tile([128, 1], F32, tag=f"wpcv{mt}", name=f"wpcv{mt}")
                   for mt in range(CT)]

        with (
            tc.tile_pool(name="stat", bufs=1) as stat,
            tc.tile_pool(name="psB", bufs=1, space="PSUM") as psB,
        ):
            # ---- stage A: InstanceNorm stats from fp8 x ----
            # Preload the sqrt table (sqrt_and_others also covers
            # copy/identity/square, so stages A+B need no further table load).
            dummy = stat.tile([128, 1], F32, tag="dummy", name="dummy")
            nc.scalar.activation(out=dummy, in_=eps_sb, func=AF.Sqrt,
                                 bias=eps_sb, scale=1.0)

            # DVE: bn_stats for ct0/ct2 and ct3-h0. ACT: sum/sumsq for ct1
            # and ct3-h1. ct3 halves are combined on DVE.
            def emit_act_half(ct, h):
                src = x8s(ct)[:, h * 2048:(h + 1) * 2048]
                scr = stat.tile([128, 2048], BF16, tag="scr",
                                name=f"scrc{ct}{h}", bufs=2)
                sx = stat.tile([128, 1], F32, tag=f"sx{ct}{h}",
                               name=f"sx{ct}{h}")
                nc.scalar.activation(out=scr, in_=src, func=AF.Copy,
                                     accum_out=sx)
                scr2 = stat.tile([128, 2048], BF16, tag="scr",
                                 name=f"scrs{ct}{h}", bufs=2)
                sx2 = stat.tile([128, 1], F32, tag=f"sx2{ct}{h}",
                                name=f"sx2{ct}{h}")
                nc.scalar.activation(out=scr2, in_=src, func=AF.Square,
                                     accum_out=sx2)
                return sx, sx2

            def emit_dve_stats(ct, nchunks=8, name=""):
                stats = stat.tile([128, nchunks, 6], F32, tag=f"stats{name}",
                                  name=f"stats{ct}{name}", bufs=2)
                for sg in range(nchunks):
                    nc.vector.bn_stats(out=stats[:, sg, :],
                                       in_=x8s(ct)[:, sg * 512:(sg + 1) * 512])
                mv = stat.tile([128, 2], F32, tag=f"mv{ct}{name}",
                               name=f"mv{ct}{name}")
                nc.vector.bn_aggr(out=mv, in_=stats)
                return mv

            def emit_combine_acts(ct, accs):
                # mu = (sx0+sx1)/HW ; var = (sx20+sx21)/HW - mu^2
                (sx0, sx20), (sx1, sx21) = accs
                mv = stat.tile([128, 2], F32, tag=f"mv{ct}", name=f"mv{ct}")
                mu = mv[:, 0:1]
                var = mv[:, 1:2]
                nc.vector.tensor_add(mu, sx0, sx1)
                nc.vector.tensor_scalar_mul(mu, mu, 1.0 / HW)
                nc.vector.tensor_add(var, sx20, sx21)
                nc.vector.tensor_scalar_mul(var, var, 1.0 / HW)
                mu2 = stat.tile([128, 1], F32, tag=f"mu2{ct}", name=f"mu2{ct}")
                nc.vector.tensor_mul(mu2, mu, mu)
                nc.vector.tensor_sub(var, var, mu2)
                return mv

            def emit_combine_half(ct, mvh, acc):
                # half-stats (mu0,var0 over h0) + ACT accums over h1
                sx1, sx21 = acc
                mv = stat.tile([128, 2], F32, tag=f"mv{ct}", name=f"mv{ct}")
                mu = mv[:, 0:1]
                var = mv[:, 1:2]
                # mu = mu0/2 + sx1/HW
                nc.vector.tensor_scalar_mul(mu, mvh[:, 0:1], 0.5)
                t1 = stat.tile([128, 1], F32, tag=f"t1{ct}", name=f"t1{ct}")
                nc.vector.tensor_scalar_mul(t1, sx1, 1.0 / HW)
                nc.vector.tensor_add(mu, mu, t1)
                # E2 = (var0 + mu0^2)/2 + sx21/HW ; var = E2 - mu^2
                t2 = stat.tile([128, 1], F32, tag=f"t2{ct}", name=f"t2{ct}")
                nc.vector.tensor_mul(t2, mvh[:, 0:1], mvh[:, 0:1])
                nc.vector.tensor_add(t2, t2, mvh[:, 1:2])
                nc.vector.tensor_scalar_mul(t2, t2, 0.5)
                nc.vector.tensor_scalar_mul(var, sx21, 1.0 / HW)
                nc.vector.tensor_add(var, var, t2)
                nc.vector.tensor_mul(t1, mu, mu)
                nc.vector.tensor_sub(var, var, t1)
                return mv

            rstd_sb = [None] * CT

            def emit_sqrt(ct, mv):
                std = stat.tile([128, 1], F32, tag=f"std{ct}", name=f"std{ct}")
                nc.scalar.activation(out=std, in_=mv[:, 1:2], func=AF.Sqrt,
                                     bias=eps_sb, scale=1.0)
                return std

            def emit_recip(ct, mv, std):
                rstd = stat.tile([128, 1], F32, tag=f"rstd{ct}",
                                 name=f"rstd{ct}")
                nc.vector.reciprocal(out=rstd, in_=std)
                rstd_sb[ct] = rstd
                # negmu8 = -8*mu (x8 keeps fp8 normal; compensated by
                # scale=1/8 in the bias drains)
                nmu = stat.tile([128, 1], F32, tag=f"nmu{ct}", name=f"nmu{ct}")
                nc.vector.tensor_scalar_mul(nmu, mv[:, 0:1], -8.0)
                nc.vector.tensor_copy(negmu8[ct // 2][:, ct % 2, :], nmu)

            def ws_dve(wn, g, j):
                nc.vector.tensor_scalar_mul(
                    ws8[wn][g][:, j, :], w_p[wn][:, g, j, :],
                    rstd_sb[2 * g + j])

            def ws_act(wn, g, j):
                nc.scalar.activation(
                    out=ws8[wn][g][:, j, :], in_=w_p[wn][:, g, j, :],
                    func=AF.Copy, scale=rstd_sb[2 * g + j])

            # ACT queue: ct1 accums -> sqrt0 -> ct3h1 accums -> sqrt2 ->
            #            sqrt1 -> sqrt3 -> ws_k j1 -> ws_q -> ws_v j1
            # DVE queue: ct0 bn -> ct2 bn -> ct3h0 bn -> combines ->
            #            recips/negmu -> ws_k j0 -> ws_v j0
            acc1h0 = emit_act_half(1, 0)
            acc1h1 = emit_act_half(1, 1)
            mv0 = emit_dve_stats(0)
            std0 = emit_sqrt(0, mv0)
            mv2 = emit_dve_stats(2)
            acc3h1 = emit_act_half(3, 1)
            std2 = emit_sqrt(2, mv2)
            mv3h = emit_dve_stats(3, nchunks=4, name="h")
            mv1 = emit_combine_acts(1, (acc1h0, acc1h1))
            std1 = emit_sqrt(1, mv1)
            mv3 = emit_combine_half(3, mv3h, acc3h1)
            std3 = emit_sqrt(3, mv3)
            emit_recip(0, mv0, std0)
            emit_recip(1, mv1, std1)
            emit_recip(2, mv2, std2)
            emit_recip(3, mv3, std3)
            ws_dve("wk", 0, 0)
            ws_act("wk", 0, 1)
            ws_dve("wk", 1, 0)
            ws_act("wk", 1, 1)
            ws_act("wq", 0, 0)
            ws_act("wq", 0, 1)
            ws_act("wq", 1, 0)
            ws_act("wq", 1, 1)
            ws_dve("wv", 0, 0)
            ws_act("wv", 0, 1)
            ws_dve("wv", 1, 0)
            ws_act("wv", 1, 1)
            # switch ACT to the exp table now (it also covers copy/identity,
            # so every later ACT op needs no further table load)
            dummy2 = stat.tile([128, 1], F32, tag="dummy", name="dummy2")
            nc.scalar.activation(out=dummy2, in_=eps_sb, func=AF.Exp,
                                 bias=eps_sb, scale=1.0)

            # ---- stage B: fp8 DR projections ----
            def act_copy(dst, src):
                nc.scalar.activation(out=dst, in_=src, func=AF.Copy)

            # GpSimd cannot read PSUM; drains go to DVE/ACT only
            drain_cycle = [nc.vector.tensor_copy, act_copy]
            drain_i = [0]

            def next_drain():
                e = drain_cycle[drain_i[0] % 2]
                drain_i[0] += 1
                return e

            # k: no bias (softmax-invariant). 64 MMs, 16 wide drains.
            for ct2 in range(CT):
                g2, j2 = ct2 // 2, ct2 % 2
                for npr in range(4):
                    ps = psB.tile([128, 1024], F32, tag="psB", bufs=3,
                                  name=f"psk{ct2}_{npr}")
                    for g in range(2):
                        for h in range(2):
                            n = 2 * npr + h
                            nc.tensor.matmul(
                                ps[:, h * 512:(h + 1) * 512],
                                ws8["wk"][g][:, :, ct2 * 128:(ct2 + 1) * 128],
                                x8_sb[:, g, :, n * 512:(n + 1) * 512],
                                start=(g == 0), stop=(g == 1), perf_mode=DR)
                    next_drain()(
                        k_sb[g2][:, j2, npr * 1024:(npr + 1) * 1024], ps)

            # q bias: qbias[ct2] = 8*bq + ws8_q @ (-mu)   (= 8*(bq + wq@nmb))
            for ct2 in range(CT):
                psb = psB.tile([128, 1], F32, tag="psBb", bufs=2,
                               name=f"psqb{ct2}")
                for g in range(2):
                    nc.tensor.matmul(
                        psb, ws8["wq"][g][:, :, ct2 * 128:(ct2 + 1) * 128],
                        negmu8[g], start=(g == 0), stop=(g == 1), perf_mode=DR)
                nc.scalar.activation(out=qbias_sb[ct2], in_=psb,
                                     func=AF.Identity,
                                     bias=bq8_sb[:, ct2:ct2 + 1], scale=0.125)

            # q: 32 MMs, 8 biased drains
            for ct2 in range(CT):
                g2, j2 = ct2 // 2, ct2 % 2
                for npr in range(2):
                    ps = psB.tile([128, 1024], F32, tag="psB", bufs=3,
                                  name=f"psq{ct2}_{npr}")
                    for g in range(2):
                        for h in range(2):
                            n = 2 * npr + h
                            nc.tensor.matmul(
                                ps[:, h * 512:(h + 1) * 512],
                                ws8["wq"][g][:, :, ct2 * 128:(ct2 + 1) * 128],
                                x8_sb[:, g, :, n * 512:(n + 1) * 512],
                                start=(g == 0), stop=(g == 1), perf_mode=DR)
                    dst = q_sb[g2][:, j2, npr * 1024:(npr + 1) * 1024]
                    if (ct2 + npr) % 2 == 0:
                        nc.scalar.activation(out=dst, in_=ps, func=AF.Identity,
                                             bias=qbias_sb[ct2], scale=1.0)
                    else:
                        nc.vector.tensor_scalar_add(dst, ps, qbias_sb[ct2])

            # v: vT[jtp] = [j=256-pair, c=512]; 64 MMs, 16 wide drains
            for jtp in range(JP):
                ps = psB.tile([128, 1024], F32, tag="psB", bufs=3,
                              name=f"psv{jtp}")
                for m in range(2):
                    jt = 2 * jtp + m
                    for g in range(2):
                        nc.tensor.matmul(
                            ps[:, m * 512:(m + 1) * 512],
                            x8_sb[:, g, :, jt * 128:(jt + 1) * 128],
                            ws8["wv"][g],
                            start=(g == 0), stop=(g == 1), perf_mode=DR)
                next_drain()(vT_sb[jtp], ps)

            # wpcv[mt] = (wp@wv) @ nmb = ws8_v' @ (-mu) / 8  (the shipped
            # "wv" is the host-folded wp@wv, so its scaled fp8 copy carries
            # rstd already); 64x from the two 8x prescales -> drain scale 1/64
            for mt in range(CT):
                psb = psB.tile([128, 1], F32, tag="psBb", bufs=2,
                               name=f"pswpcv{mt}")
                for g in range(2):
                    nc.tensor.matmul(
                        psb, ws8["wv"][g][:, :, mt * 128:(mt + 1) * 128],
                        negmu8[g], start=(g == 0), stop=(g == 1), perf_mode=DR)
                nc.scalar.activation(out=wpcv_sb[mt], in_=psb, func=AF.Copy,
                                     scale=1.0 / 64.0)

        # ---- stage C: attention per i-block. The output projection is
        # host-folded into V (the shipped "wv" is wp@wv), so U = V'@attn IS
        # the projected output: y = U*rzb + xr'. QK pairs are pipelined two
        # ahead ACROSS i-block boundaries to keep the PE fed through the
        # Z-reduce / psU-drain handoff.
        with (
            tc.tile_pool(name="expp", bufs=3) as expp,
            tc.tile_pool(name="zp", bufs=2) as zp,
            tc.tile_pool(name="xrp", bufs=4) as xrp,
            tc.tile_pool(name="yp", bufs=5) as yp,
            tc.tile_pool(name="psC", bufs=1, space="PSUM") as psC,
        ):
            def emit_qk(gidx):
                ib, jtp = divmod(gidx, JP)
                isl = slice(ib * 512, (ib + 1) * 512)
                ps = psC.tile([128, 1024], F32, tag="psL", bufs=2,
                              name=f"psL{jtp}_{ib}")
                for m in range(2):
                    jt = 2 * jtp + m
                    for g in range(2):
                        nc.tensor.matmul(
                            ps[:, m * 512:(m + 1) * 512],
                            k_sb[g][:, :, jt * 128:(jt + 1) * 128],
                            q_sb[g][:, :, isl],
                            start=(g == 0), stop=(g == 1), perf_mode=DR)
                return ps

            qk_q = [emit_qk(0), emit_qk(1)]
            xq2_emit = []

            for ib in range(IB):
                isl = slice(ib * 512, (ib + 1) * 512)
                psU = [psC.tile([128, 512], F32, tag=f"psU{ct}", bufs=1,
                                name=f"psU{ct}_{ib}") for ct in range(CT)]
                # fp16 Z partials: ~0.1% accumulation noise, but the ones-
                # matmul runs at 1 cyc/row instead of fp32's 4
                zv = zp.tile([128, 512], FP16, tag="zv", name=f"zv{ib}")
                zg = zp.tile([128, 512], FP16, tag="zg", name=f"zg{ib}")

                first_exp = None
                for jtp in range(JP):
                    ps = qk_q.pop(0)
                    ex = expp.tile([128, 2, 512], FP8, tag="expT",
                                   name=f"ex{jtp}_{ib}")
                    einst = nc.scalar.activation(out=ex, in_=ps, func=AF.Exp,
                                                 bias=expoff_sb,
                                                 scale=float(SCALE8))
                    if first_exp is None:
                        first_exp = einst
                    # previous i-block's deferred xr' ACT ops, spread out so
                    # they never form a bubble between exps
                    if jtp in (1, 4, 7, 10) and xq2_emit:
                        xq2_emit.pop(0)()
                    nxt = ib * JP + jtp + 2
                    if nxt < IB * JP:
                        qk_q.append(emit_qk(nxt))
                    # denominator partials: even pairs (and the last, so the
                    # final Z chain avoids GpSimd latency) on DVE, odd on
                    # GpSimd
                    on_dve = jtp % 2 == 0 or jtp == JP - 1
                    eng = nc.vector if on_dve else nc.gpsimd
                    acc = zv if on_dve else zg
                    if jtp < 2:
                        eng.tensor_add(acc, ex[:, 0, :], ex[:, 1, :])
                    else:
                        eng.tensor_add(acc, acc, ex[:, 0, :])
                        eng.tensor_add(acc, acc, ex[:, 1, :])
                    for ct in range(CT):
                        nc.tensor.matmul(
                            psU[ct], vT_sb[jtp][:, :, ct * 128:(ct + 1) * 128],
                            ex, start=(jtp == 0), stop=(jtp == JP - 1),
                            perf_mode=DR)

                nc.vector.tensor_add(zv, zv, zg)

                # residual prefetch (DMA gated on this block's first exp)
                xr_ts = []
                for mt in range(CT):
                    xr_t = xrp.tile([128, 512], BF16, tag="xrb",
                                    name=f"xrb{mt}_{ib}")
                    xd = nc.sync.dma_start(out=xr_t, in_=xr[:, mt, isl])
                    bass._add_dep_helper(xd.ins, first_exp.ins, sync=True,
                                         reason="delay residual load")
                    xr_ts.append(xr_t)

                # Z partition-reduce + broadcast; 1/(8Z) once per i-block
                psZ = psC.tile([128, 512], F32, tag="psL", bufs=2,
                               name=f"psZ{ib}")
                nc.tensor.matmul(psZ, ones8_sb, zv, start=True, stop=True)
                rzb = zp.tile([128, 512], F32, tag="rzb", name=f"rzb{ib}")
                nc.vector.reciprocal_approx_fast(out=rzb, in_=psZ)

                # y1 = U*rzb drains psU on DVE (must precede the next
                # i-block's EV start). The [xr' -> add -> DMA] finish is
                # deferred into the next i-block's j-loop so the four ACT
                # xr' ops never form a bubble between exps.
                for mt in range(CT):
                    y1 = yp.tile([128, 512], F32, tag="y1", name=f"y1{mt}_{ib}")
                    nc.vector.tensor_mul(y1, psU[mt], rzb)

                    def fin(y1=y1, xr_t=xr_ts[mt], mt=mt, ib=ib, isl=isl,
                            last=(ib == IB - 1)):
                        xq2 = xrp.tile([128, 512], F32, tag="xrf",
                                       name=f"xrf{mt}_{ib}")
                        nc.scalar.activation(out=xq2, in_=xr_t,
                                             func=AF.Identity,
                                             bias=wpcv_sb[mt], scale=1.0)
                        y = yp.tile([128, 512], F32, tag="y",
                                    name=f"y{mt}_{ib}")
                        if last and mt % 2 == 0:
                            nc.vector.tensor_add(y, y1, xq2)
                        else:
                            nc.gpsimd.tensor_add(y, y1, xq2)
                        nc.sync.dma_start(
                            out=out[mt * 128:(mt + 1) * 128, isl], in_=y)
                    if ib == IB - 1:
                        fin()
                    else:
                        xq2_emit.append(fin)


_NC = None


def _get_nc():
    global _NC
    if _NC is None:
        _NC = build_nc()
    return _NC


def make_in_maps(x, wq, bq, wk, bk, wv, bv, wp, bp):
    x = np.asarray(x, dtype=np.float32)
    wq, wk, wv, wp = (np.asarray(a, dtype=np.float32) for a in (wq, wk, wv, wp))
    bq, bk, bv, bp = (np.asarray(a, dtype=np.float32) for a in (bq, bk, bv, bp))
    bp2 = wp @ bv + bp

    def pack_w_pair(w):
        # [p, g, j, o] = 8 * w[o, g*256 + j*128 + p]
        return np.ascontiguousarray(
            (WS * w.T).reshape(2, 2, 128, C).transpose(2, 0, 1, 3)
        ).astype(ml_dtypes.bfloat16)

    shared = {
        "wqp": pack_w_pair(wq), "wkp": pack_w_pair(wk),
        # the output projection is folded into V: v' = (wp@wv) @ xn
        "wvp": pack_w_pair(wp @ wv),
        "bq8": np.ascontiguousarray(
            (WS * bq).reshape(CT, 128).T).astype(np.float32),
    }
    in_maps = []
    for core in range(N_CORES):
        b, h = divmod(core, 2)
        xb = x[b].reshape(C, HW)
        xc = np.roll(xb, -h * NQ, axis=1) + bp2[:, None]
        x8 = np.ascontiguousarray(
            xc.reshape(2, 2, 128, HW).transpose(2, 0, 1, 3)
        ).astype(ml_dtypes.float8_e4m3)
        xrh = np.ascontiguousarray(
            xc[:, :NQ].reshape(CT, 128, NQ).transpose(1, 0, 2)
        ).astype(ml_dtypes.bfloat16)
        in_maps.append({"x8": x8, "xr": xrh, **shared})
    return in_maps


def assemble_out(results):
    out = np.empty((B, C, HW), dtype=np.float32)
    for core in range(N_CORES):
        b, h = divmod(core, 2)
        out[b][:, h * NQ:(h + 1) * NQ] = results[core]["out"]
    return out.reshape(B, C, 64, 64)


def kernel(x, wq, bq, wk, bk, wv, bv, wp, bp):
    nc = _get_nc()
    in_maps = make_in_maps(x, wq, bq, wk, bk, wv, bv, wp, bp)
    res = bass_utils.run_bass_kernel_spmd(nc, in_maps,
                                          core_ids=list(range(N_CORES)))
    return assemble_out(res.results)
